# revision 38
# baseline (speedup 1.0000x reference)
"""Trainium2 Bass kernel for per-sample segment-mean + tiny GEMM.

Reference computation (per sample b):
    sums[w]  = segment_sum(x[b], word_ids[b])           # [512, 1024]
    cnt[w]   = segment_sum(ones, word_ids[b])           # [512]
    means    = sums / max(cnt, 1)
    out[b]   = means[word_ids[b]] @ W + b               # [2048, 3]

Key identity: means[wid]@W == (segment_sum(x@W)/cnt)[wid], so the big
[512,1024] segment-sum intermediate is never materialized.  Per core
(2 samples, 8 strips of 512 tokens):

  1. Host uploads xT in fp16 (halves HBM traffic; rel-err ~1e-3 vs the
     2e-2 gate), strip-major so each half-strip is one 0.5MB DMA of 128
     contiguous 4KB descriptors on the sync HWDGE queue.
  2. GEMM: ytT[c,t] = sum_h W[h,c] xT[h,t] accumulated over 8 h-blocks
     into a [4,512] PSUM tile per strip (W stationary [128,4] fp16).
  3. ytT -> 4 PE transposes into ONE [128,16] PSUM tile per strip (the
     first has start=True which zeroes the whole 2KB PSUM region, the
     rest accumulate onto zeros) -> one fp16 evac -> y16 [128t,4c].
  4. Segment-sum, ind-stationary: ysum[w,c] += ind_chunk.T @ y16_chunk
     straight into word-major [128, wb*4+c] PSUM (4-cycle matmuls; one
     start=True pends the whole 2KB region, later matmuls each touch
     exactly their wb sub-region so HW pending-zero bytes initialize
     each region -- no transposes needed afterwards).
  5. DVE: *rec (host-built 1/max(cnt,1)) then +bias -> ymean fp16,
     reading the PSUM accumulator directly.
  6. Gather-back per strip: outT[c,t] = sum_wb ymean[wb] @ indT[wb]
     (indT = is_equal(wib, iotap) on DVE from a PE ones-broadcast of
     the word-id row, narrowed to each word block's token span; bias
     lands exactly once per token).

Scheduling notes (each cost ~2-10us when violated):
  - HAM: the PE's default clock state is throttled 4/8 (1.2GHz); only
    sustained activity releases it to 8/8.  Five f32 warm-up matmuls
    fill the first activity window while DMA streams in.
  - Per-instruction overhead (LDWEIGHTS + NX issue + semaphores,
    ~100-250ns) dominates small matmuls, so the instruction count is
    kept to ~170 (vs 256+ in earlier variants).
  - Const DMA *issue* instructions cost ~650ns of engine time each;
    late-needed consts are issued mid-stream so they don't delay the
    first strip's PSUM evacuations on the scalar queue.
  - wib/indT generation and sample-0 gathers are woven between strips
    to keep the PE dense (HAM) without blocking the strip pipeline.
  - Indirect-DMA token gather does NOT work on HW: the DGE honors one
    offset per partition and streams consecutive rows (verified by
    decoding the permutation), unlike CoreSim's multi-index model.
"""

import numpy as np

import concourse.bass as bass
import concourse.bacc as bacc
import concourse.mybir as mybir
import concourse.tile as tile
from concourse.bass_utils import run_bass_kernel_spmd
from concourse.masks import make_identity

B, S, H, C = 16, 2048, 1024, 3
NW = 512
P = 128
N_CORES = 8
SPC = B // N_CORES          # samples per core
NCH = S // P                # 128-token chunks per sample (16)
NST = S // 512              # 512-token strips per sample (4)
NWB = NW // P               # word blocks (4)
NHB = H // P                # h blocks (8)
F32 = mybir.dt.float32
F16 = mybir.dt.float16

_CACHE = {}
TRACE = False          # set by test harness to capture an NTFF profile
LAST_RESULTS = None    # BassKernelResults of the most recent run


def _build_maps(word_ids):
    """Per-chunk / per-strip touched word-block spans plus per-(strip,wb)
    token sub-spans, unioned across all samples so the same program is
    valid on every core."""
    chunk_wbs = [set() for _ in range(NCH)]
    strip_wbs = [set() for _ in range(NST)]
    tok_span = {}
    for bi in range(B):
        for ci in range(NCH):
            seg = word_ids[bi, ci * P:(ci + 1) * P]
            lo, hi = int(seg.min()) // P, int(seg.max()) // P
            chunk_wbs[ci].update(range(lo, hi + 1))
        for si in range(NST):
            seg = word_ids[bi, si * 512:(si + 1) * 512]
            wbs = seg // P
            lo, hi = int(wbs.min()), int(wbs.max())
            strip_wbs[si].update(range(lo, hi + 1))
            for wb in range(lo, hi + 1):
                import numpy as _np
                pos = _np.nonzero(wbs == wb)[0]
                if len(pos) == 0:
                    continue
                t0, t1 = int(pos[0]), int(pos[-1]) + 1
                if (si, wb) in tok_span:
                    a, b = tok_span[(si, wb)]
                    tok_span[(si, wb)] = (min(a, t0), max(b, t1))
                else:
                    tok_span[(si, wb)] = (t0, t1)
    chunk_wbs = [sorted(s) for s in chunk_wbs]
    strip_wbs = [sorted(s) for s in strip_wbs]
    # ensure every (si, wb) in the union has a span (may be absent for a
    # sample but present for another; union handled above)
    return chunk_wbs, strip_wbs, tuple(sorted(tok_span.items()))


def _build_program(maps):
    chunk_wbs, strip_wbs, tok_span_items = maps
    tok_span = dict(tok_span_items)
    n_indt = SPC * sum(len(sw) for sw in strip_wbs)
    n_pa = sum(len(cw) for cw in chunk_wbs)

    nc = bacc.Bacc(
        "TRN2",
        target_bir_lowering=False,
        debug=False,
        enable_asserts=False,
        num_devices=N_CORES,
    )

    TSTR = SPC * NST            # strips per core (8)
    xt = nc.dram_tensor("xt", [TSTR * P, NHB * 512], F16,
                        kind="ExternalInput").ap()
    wt16 = nc.dram_tensor("wt16", [P, NHB * 4], F16, kind="ExternalInput").ap()
    iota16 = nc.dram_tensor("iota16", [P, NW], F16, kind="ExternalInput").ap()
    wic32 = nc.dram_tensor("wic32", [P, SPC * NCH], F32,
                           kind="ExternalInput").ap()
    recb = nc.dram_tensor("recb", [P, SPC * NWB * 4], F32,
                          kind="ExternalInput").ap()
    bbt = nc.dram_tensor("bbt", [P, NWB * 4], F32, kind="ExternalInput").ap()
    iotap = nc.dram_tensor("iotap", [P, NWB], F32, kind="ExternalInput").ap()
    wir16 = nc.dram_tensor("wir16", [1, SPC * S], F16,
                           kind="ExternalInput").ap()
    ones16 = nc.dram_tensor("ones16", [1, P], F16, kind="ExternalInput").ap()
    yout = nc.dram_tensor("yout", [SPC, C, S], F32, kind="ExternalOutput").ap()

    with tile.TileContext(nc) as tc:
        with (
            tc.tile_pool(name="pp_yt", bufs=2, space="PSUM") as pp_yt,
            tc.tile_pool(name="pp_small", bufs=2, space="PSUM") as pp_small,
            tc.tile_pool(name="pp_ysum", bufs=2, space="PSUM") as pp_ysum,
            tc.tile_pool(name="pp_gat", bufs=2, space="PSUM") as pp_gat,
            tc.tile_pool(name="pl_x", bufs=14) as pl_x,
            tc.tile_pool(name="pl_ind", bufs=SPC * NCH) as pl_ind,
            tc.tile_pool(name="pl_y16", bufs=4) as pl_y16,
            tc.tile_pool(name="pl_ytT", bufs=4) as pl_ytT,
            tc.tile_pool(name="pl_ys", bufs=4) as pl_ys,
            tc.tile_pool(name="pl_indT", bufs=n_indt) as pl_indT,
            tc.tile_pool(name="pl_out", bufs=4) as pl_out,
            tc.tile_pool(name="pl_const", bufs=1) as pl_const,
        ):
            # ---- x strip DMAs (two 0.5MB halves per strip for finer
            # DMA/PE pacing), all queued up front on the sync HWDGE ----
            HLF = NHB // 2 * 512
            x_tiles = []
            for st in range(SPC * NST):
                ta = pl_x.tile([P, HLF], F16, tag="xa", name=f"xa_{st}")
                if st == 0:
                    # first strip in quarters: GEMM starts ~0.7us earlier
                    nc.sync.dma_start(out=ta[:, 0:HLF // 2],
                                      in_=xt[0:P, 0:HLF // 2])
                    nc.sync.dma_start(out=ta[:, HLF // 2:HLF],
                                      in_=xt[0:P, HLF // 2:HLF])
                else:
                    nc.sync.dma_start(out=ta[:], in_=xt[st * P:(st + 1) * P,
                                                        0:HLF])
                tb = pl_x.tile([P, HLF], F16, tag="xb", name=f"xb_{st}")
                nc.sync.dma_start(out=tb[:], in_=xt[st * P:(st + 1) * P,
                                                    HLF:2 * HLF])
                x_tiles.append((ta, tb))

            # ---- constants on the scalar HWDGE queue ----
            wt_sb = pl_const.tile([P, NHB * 4], F16, tag="wt")
            nc.scalar.dma_start(out=wt_sb[:], in_=wt16[:])
            iota_sb = pl_const.tile([P, NW], F16, tag="iota")
            nc.scalar.dma_start(out=iota_sb[:], in_=iota16[:])
            wic_sb = pl_const.tile([P, SPC * NCH], F32, tag="wic")
            nc.scalar.dma_start(out=wic_sb[:], in_=wic32[:])
            ident = pl_const.tile([P, P], F32, tag="ident")
            make_identity(nc, ident[:])
            # preload the activation table so the first real scalar.copy
            # isn't stalled 1.3us behind ACT_TABLE_LOAD
            dummy = pl_const.tile([1, 1], F32, tag="dummy")
            nc.scalar.copy(out=dummy[:], in_=ident[0:1, 0:1])
            # deferred consts (allocated now, DMA'd inside the strip loop so
            # their issue cost doesn't clog the scalar queue ahead of the
            # first strip evacuations)
            recb_sb = pl_const.tile([P, SPC * NWB * 4], F32, tag="recb")
            bbt_sb = pl_const.tile([P, NWB * 4], F32, tag="bbt")
            iotap_sb = pl_const.tile([P, NWB], F32, tag="iotap")
            wir_sb = pl_const.tile([1, SPC * S], F16, tag="wir")
            ones_sb = pl_const.tile([1, P], F16, tag="ones")

            # ---- PE warm-up: HAM's default state is throttled (1.2GHz);
            # ~3.4us of sustained activity releases it to 2.4GHz.  Three
            # f32 matmuls (4 cyc/row) fill the window cheaply.
            warm = pp_gat.tile([P, P], F32, tag="gat", name="warm")
            for r in range(5):
                nc.tensor.matmul(
                    out=warm[:],
                    lhsT=ident[:],
                    rhs=ident[:],
                    start=True, stop=True,
                )

            indT_sb = {}

            def make_indT(st):
                s, si = st // NST, st % NST
                wib = pp_small.tile([P, 512], F32, tag="sm",
                                    name=f"wib_{s}_{si}")
                nc.tensor.matmul(
                    out=wib[:],
                    lhsT=ones_sb[:],
                    rhs=wir_sb[0:1,
                               s * S + si * 512:s * S + (si + 1) * 512],
                    start=True, stop=True,
                )
                for j, wb in enumerate(strip_wbs[si]):
                    t0, t1 = (0, 512) if j == 0 else tok_span[(si, wb)]
                    it = pl_indT.tile([P, 512], F16, tag="indT",
                                      name=f"indT_{s}_{si}_{wb}")
                    nc.vector.tensor_scalar(
                        out=it[:, 0:t1 - t0],
                        in0=wib[:, t0:t1],
                        scalar1=iotap_sb[:, wb:wb + 1],
                        scalar2=None,
                        op0=mybir.AluOpType.is_equal,
                    )
                    indT_sb[(s, si, wb)] = it

            # ---- chunk indicators ind[(s,ci)] on DVE ----
            # chunk 0 is full-width so its phase-A matmul (start=True)
            # zeroes the whole [4,512] ysumT accumulator region.
            ind_sb = {}
            for s in range(SPC):
                for ci in range(NCH):
                    lo, hi = chunk_wbs[ci][0], chunk_wbs[ci][-1]
                    t = pl_ind.tile([P, NW], F16, tag="ind",
                                    name=f"ind_{s}_{ci}")
                    nc.vector.tensor_scalar(
                        out=t[:, 0:(hi - lo + 1) * P],
                        in0=iota_sb[:, lo * P:(hi + 1) * P],
                        scalar1=wic_sb[:, s * NCH + ci:s * NCH + ci + 1],
                        scalar2=None,
                        op0=mybir.AluOpType.is_equal,
                    )
                    ind_sb[(s, ci)] = (t, lo, hi)

            # ---- per-strip work, software-pipelined by one strip ----
            ysum_t = {}
            pa_cnt = {}

            def gemm(st):
                yt = pp_yt.tile([4, 512], F32, tag="yt", name=f"yt_{st}")
                for hb in range(NHB):
                    half = x_tiles[st][hb // (NHB // 2)]
                    off = (hb % (NHB // 2)) * 512
                    nc.tensor.matmul(
                        out=yt[:],
                        lhsT=wt_sb[:, hb * 4:(hb + 1) * 4],
                        rhs=half[:, off:off + 512],
                        start=(hb == 0),
                        stop=(hb == NHB - 1),
                    )
                hb_t = pp_gat.tile([P, P], F32, tag="gat", name=f"hb_{st}")
                nc.tensor.matmul(out=hb_t[:], lhsT=ones_sb[:],
                                 rhs=ones_sb[:], start=True, stop=True)
                return yt

            def reduce_strip(st, yt):
                s, si = st // NST, st % NST
                ytT = pl_ytT.tile([4, 512], F32, tag="ytT", name=f"ytT_{st}")
                nc.scalar.copy(out=ytT[:], in_=yt[:])
                ty = pp_small.tile([P, 16], F32, tag="sm", name=f"ty_{st}")
                for k in range(4):
                    nc.tensor.matmul(
                        out=ty[:, k * 4:(k + 1) * 4],
                        lhsT=ytT[:, k * P:(k + 1) * P],
                        rhs=ident[0:4, 0:4],
                        is_transpose=True,
                        start=(k == 0),
                        stop=(k == 3),
                    )
                y16 = pl_y16.tile([P, 16], F16, tag="y16", name=f"y16_{st}")
                nc.scalar.copy(out=y16[:], in_=ty[:])
                # phase A: ysum[w, c] += ind_chunk.T @ y16_chunk, straight
                # into word-major [128, 4*wb+c] layout (no ys2 transpose).
                # One start=True pends the whole 2KB region; each later
                # matmul touches exactly its wb sub-region, so the HW
                # pending-zero bytes initialize each region correctly.
                if si == 0:
                    ysum_t[s] = pp_ysum.tile([P, NWB * 4], F32, tag="ysum",
                                             name=f"ysum_{s}")
                    pa_cnt[s] = 0
                for k in range(4):
                    ci = si * 4 + k
                    ind_t, lo, hi = ind_sb[(s, ci)]
                    for wb in chunk_wbs[ci]:
                        pa_cnt[s] += 1
                        nc.tensor.matmul(
                            out=ysum_t[s][:, wb * 4:(wb + 1) * 4],
                            lhsT=ind_t[:, (wb - lo) * P:(wb - lo + 1) * P],
                            rhs=y16[:, k * 4:(k + 1) * 4],
                            start=(pa_cnt[s] == 1),
                            stop=(pa_cnt[s] == n_pa),
                        )

            ym16_t = {}

            def finish_means(s):
                # means = ysum * rec + bias, read straight from PSUM
                ym = pl_ys.tile([P, NWB * 4], F32, tag="ym", name=f"ym_{s}")
                nc.vector.tensor_tensor(
                    out=ym[:],
                    in0=ysum_t[s][:],
                    in1=recb_sb[:, s * NWB * 4:(s + 1) * NWB * 4],
                    op=mybir.AluOpType.mult,
                )
                ym16 = pl_ys.tile([P, NWB * 4], F16, tag="ym16",
                                  name=f"ym16_{s}")
                nc.vector.tensor_tensor(
                    out=ym16[:],
                    in0=ym[:],
                    in1=bbt_sb[:],
                    op=mybir.AluOpType.add,
                )
                ym16_t[s] = ym16

            def gather_strip(s, si):
                # outT[c,t] = sum_w ym16[w,c] indT[w,t]; bias lands exactly
                # once since each token hits exactly one word
                ym16 = ym16_t[s]
                outT = pp_gat.tile([4, 512], F32, tag="gat",
                                   name=f"outT_{s}_{si}")
                for j, wb in enumerate(strip_wbs[si]):
                    t0, t1 = (0, 512) if j == 0 else tok_span[(si, wb)]
                    nc.tensor.matmul(
                        out=outT[:, t0:t1],
                        lhsT=ym16[:, wb * 4:(wb + 1) * 4],
                        rhs=indT_sb[(s, si, wb)][:, 0:t1 - t0],
                        start=(j == 0),
                        stop=(j == len(strip_wbs[si]) - 1),
                    )
                os_t = pl_out.tile([4, 512], F32, tag="out",
                                   name=f"out_{s}_{si}")
                (nc.scalar.copy if (s == 0 or si % 2 == 0)
                 else nc.vector.tensor_copy)(out=os_t[:], in_=outT[:])
                nc.sync.dma_start(
                    out=yout[s][:, si * 512:(si + 1) * 512],
                    in_=os_t[0:C, :],
                )

            def deferred_consts(st):
                # spread the deferred const DMA issues across early strips
                if st == 0:
                    nc.scalar.dma_start(out=iotap_sb[:], in_=iotap[:])
                    nc.scalar.dma_start(out=ones_sb[:], in_=ones16[:])
                    nc.scalar.dma_start(out=wir_sb[:], in_=wir16[:])
                elif st == 1:
                    nc.scalar.dma_start(out=recb_sb[:], in_=recb[:])
                elif st == 2:
                    nc.scalar.dma_start(out=bbt_sb[:], in_=bbt[:])

            prev = None
            for st in range(SPC * NST):
                yt = gemm(st)
                if prev is not None:
                    pst = prev[0]
                    reduce_strip(*prev)
                    deferred_consts(pst)
                    make_indT(pst)
                    if pst == NST - 1:
                        finish_means(0)
                    elif pst >= NST:
                        gather_strip(0, pst - NST)
                prev = (st, yt)
            reduce_strip(*prev)
            make_indT(prev[0])
            gather_strip(0, NST - 1)
            finish_means(1)
            for si in range(NST):
                gather_strip(1, si)

    nc.compile()
    return nc


def core_inputs(x, word_ids, W, b):
    """Host-side prep: per-core input maps (shared by kernel and tests)."""
    x = np.ascontiguousarray(np.asarray(x, dtype=np.float32))
    word_ids = np.asarray(word_ids, dtype=np.int32)
    W = np.asarray(W, dtype=np.float32)
    b = np.asarray(b, dtype=np.float32)

    iota16 = np.broadcast_to(np.arange(NW, dtype=np.float16),
                             (P, NW)).copy()
    iotap = (np.arange(P, dtype=np.float32)[:, None]
             + P * np.arange(NWB, dtype=np.float32)[None, :]).copy()
    ones16 = np.ones((1, P), dtype=np.float16)
    wt16 = np.zeros((P, NHB * 4), dtype=np.float16)
    for hb in range(NHB):
        wt16[:, hb * 4:hb * 4 + C] = W[hb * P:(hb + 1) * P, :]
    bbt = np.zeros((P, NWB * 4), dtype=np.float32)
    for j in range(NWB):
        bbt[:, j * 4:j * 4 + C] = b[None, :]

    # counts -> reciprocals per (sample, word)
    cnt = np.zeros((B, NW), dtype=np.float32)
    for bi in range(B):
        cnt[bi] = np.bincount(word_ids[bi], minlength=NW)
    recf = 1.0 / np.maximum(cnt, 1.0)                      # [B, NW]

    x16 = x.astype(np.float16)
    in_maps = []
    for core in range(N_CORES):
        sl = slice(core * SPC, (core + 1) * SPC)
        xc = x16[sl]                                       # [SPC, S, H]
        # xt[st*128+p, hb*512+t] = x[s, st0*512+t, hb*128+p]
        xtc = (xc.reshape(SPC * NST, 512, NHB, P)
               .transpose(0, 3, 2, 1)                      # [str, p, hb, t]
               .reshape(SPC * NST * P, NHB * 512))
        xtc = np.ascontiguousarray(xtc)

        wi_core = word_ids[sl]                             # [SPC, S] int32
        wic32 = np.zeros((P, SPC * NCH), dtype=np.float32)
        for s in range(SPC):
            for ci in range(NCH):
                wic32[:, s * NCH + ci] = wi_core[s, ci * P:(ci + 1) * P]

        # recb[p, s*16 + j*4 + c] = rec[sample s, word j*128+p]
        recb = np.zeros((P, SPC * NWB * 4), dtype=np.float32)
        for s in range(SPC):
            r = recf[core * SPC + s].reshape(NWB, P).T      # [P, NWB]
            recb[:, s * NWB * 4:(s + 1) * NWB * 4] = np.repeat(r, 4, axis=1)

        in_maps.append({
            "xt": xtc,
            "wt16": wt16,
            "iota16": iota16,
            "wic32": wic32,
            "recb": recb,
            "bbt": bbt,
            "iotap": iotap,
            "wir16": wi_core.astype(np.float16).reshape(1, -1).copy(),
            "ones16": ones16,
        })
    return in_maps


def kernel(x, word_ids, W, b):
    word_ids = np.asarray(word_ids, dtype=np.int32)
    maps = _build_maps(word_ids)
    key = repr(maps)
    if key not in _CACHE:
        _CACHE[key] = _build_program(maps)
    nc = _CACHE[key]

    in_maps = core_inputs(x, word_ids, W, b)

    global LAST_RESULTS
    res = run_bass_kernel_spmd(nc, in_maps, list(range(N_CORES)), trace=TRACE)
    LAST_RESULTS = res
    out = np.empty((B, S, C), dtype=np.float32)
    for core in range(N_CORES):
        yc = res.results[core]["yout"]                      # [SPC, C, S]
        out[core * SPC:(core + 1) * SPC] = yc.transpose(0, 2, 1)
    return out
